# revision 9
# baseline (speedup 1.0000x reference)
"""Multi-head attention (B=2, S=2048, DIM=1024, H=16) on 8 Trainium2 cores.

Sharding: data-parallel over batch x tensor-parallel over heads.
Core c handles batch c//4 and heads 4*(c%4) .. 4*(c%4)+4.
Each core computes q/k/v projections for its 256 features, causal
attention for its 4 heads (writing the normalized attention
probabilities), and a partial output projection; the host sums the four
partial outputs per batch and adds the (host-folded) biases.
"""

import sys

sys.path.insert(0, "/opt/trn_rl_repo")

import functools

import numpy as np

B, S, DIM, H = 2, 2048, 1024, 16
HD = DIM // H  # 64
NCORES = 8
GROUPS = NCORES // B  # 4 head-groups per batch
HPC = H // GROUPS  # 4 heads per core
FPC = HPC * HD  # 256 features per core
P = 128
QB = 512  # q-block (attention inner block of 4 q-tiles)
SBLK = 256  # phase-1 seq block


@functools.lru_cache(maxsize=2)
def _build(causal: bool):
    import concourse.bass as bass
    import concourse.mybir as mybir
    import concourse.tile as tile
    from concourse import bacc
    from concourse.masks import make_causal_mask, make_identity

    f32 = mybir.dt.float32
    f32r = mybir.dt.float32r
    AF = mybir.ActivationFunctionType

    nc = bacc.Bacc()
    xq = nc.declare_dram_parameter("xq", [S, DIM], f32, isOutput=False)
    xk = nc.declare_dram_parameter("xk", [S, DIM], f32, isOutput=False)
    xv = nc.declare_dram_parameter("xv", [S, DIM], f32, isOutput=False)
    wq = nc.declare_dram_parameter("wq", [DIM, FPC], f32, isOutput=False)
    wk = nc.declare_dram_parameter("wk", [DIM, FPC], f32, isOutput=False)
    wv = nc.declare_dram_parameter("wv", [DIM, FPC], f32, isOutput=False)
    wo = nc.declare_dram_parameter("wo", [FPC, DIM], f32, isOutput=False)
    bqk = nc.declare_dram_parameter("bqk", [P, 4], f32, isOutput=False)
    if not causal:
        amask = nc.declare_dram_parameter("amask", [S, S], f32, isOutput=False)
    attn_d = nc.declare_dram_parameter("attn", [HPC, S, S], f32, isOutput=True)
    out_d = nc.declare_dram_parameter("out_part", [S, DIM], f32, isOutput=True)

    KC = DIM // P  # 8 fin chunks
    NST = S // P  # 16 seq tiles
    n_qb = S // QB  # 4

    with tile.TileContext(nc) as tc:
        with (
            tc.tile_pool(name="const", bufs=1) as const,
            tc.tile_pool(name="persist", bufs=1) as persist,
            tc.tile_pool(name="xload", bufs=2) as xload,
            tc.tile_pool(name="xtp", bufs=2) as xtp,
            tc.tile_pool(name="attp", bufs=1) as attp,
            tc.tile_pool(name="attrow", bufs=2) as attrow,
            tc.tile_pool(name="small", bufs=4) as small,
            tc.tile_pool(name="aop", bufs=1) as aop,
            tc.tile_pool(name="osbp", bufs=2) as osbp,
            tc.tile_pool(name="psA", bufs=3, space="PSUM") as psA,
            tc.tile_pool(name="psB", bufs=3, space="PSUM") as psB,
            tc.tile_pool(name="psC", bufs=2, space="PSUM") as psC,
        ):
            ident = const.tile([P, P], f32)
            make_identity(nc, ident[:])
            zeros = const.tile([P, S // 2], f32)
            nc.gpsimd.memset(zeros[:], 0.0)
            if causal:
                cmask = const.tile([P, P], f32)
                make_causal_mask(nc, cmask[:], mask_val=-1e9)

            # persistent activations (float32r, matmul-ready)
            qT = persist.tile([P, 2, S], f32r)  # [fout_part, fout_chunk, seq]
            kT = persist.tile([P, 2, S], f32r)
            v_sb = persist.tile([P, NST, FPC], f32r)  # [s_part, s_tile, fout]
            wq_r = persist.tile([P, KC, FPC], f32r)
            wk_r = persist.tile([P, KC, FPC], f32r)
            wv_r = persist.tile([P, KC, FPC], f32r)
            wo_r = persist.tile([64, 4, DIM], f32r)
            bqk_sb = persist.tile([P, 4], f32)
            nc.sync.dma_start(bqk_sb[:], bqk[:])

            # round weights to f32r once
            for w_dram, w_dst in ((wq, wq_r), (wk, wk_r), (wv, wv_r)):
                wtmp = xload.tile([P, KC, FPC], f32, tag="xt")
                nc.sync.dma_start(
                    wtmp[:], w_dram[:].rearrange("(c p) f -> p c f", p=P)
                )
                nc.vector.tensor_copy(w_dst[:], wtmp[:])
            wotmp = xload.tile([64, 4, DIM], f32, tag="xt")
            nc.sync.dma_start(
                wotmp[:], wo[:].rearrange("(c p) f -> p c f", p=64)
            )
            nc.vector.tensor_copy(wo_r[:], wotmp[:])

            # ---- phase 1: projections ----
            for blk in range(S // SBLK):
                s0 = blk * SBLK
                for which, (x_d, w_r) in enumerate(
                    ((xq, wq_r), (xk, wk_r), (xv, wv_r))
                ):
                    xt = xload.tile([P, SBLK // P, DIM], f32, tag="xt")
                    nc.sync.dma_start(
                        xt[:],
                        x_d[s0 : s0 + SBLK, :].rearrange("(t p) f -> p t f", p=P),
                    )
                    xT = xtp.tile([P, KC, SBLK], f32r, tag="xT")
                    for fc in range(KC):
                        for st in range(SBLK // P):
                            pt = psB.tile([P, P], f32, tag="pt")
                            nc.tensor.transpose(
                                pt[:], xt[:, st, fc * P : (fc + 1) * P], ident[:]
                            )
                            nc.vector.tensor_copy(
                                xT[:, fc, st * P : (st + 1) * P], pt[:]
                            )
                    if which < 2:  # q, k -> transposed layout + bias
                        dst = qT if which == 0 else kT
                        for m in range(2):
                            pq = psA.tile([P, QB], f32, tag="A")
                            for kc in range(KC):
                                nc.tensor.matmul(
                                    pq[:, :SBLK],
                                    w_r[:, kc, m * P : (m + 1) * P],
                                    xT[:, kc, :],
                                    start=(kc == 0),
                                    stop=(kc == KC - 1),
                                )
                            nc.vector.tensor_scalar_add(
                                dst[:, m, s0 : s0 + SBLK],
                                pq[:, :SBLK],
                                bqk_sb[:, 2 * which + m : 2 * which + m + 1],
                            )
                    else:  # v -> natural layout
                        for st in range(SBLK // P):
                            pv = psA.tile([P, QB], f32, tag="A")
                            for kc in range(KC):
                                nc.tensor.matmul(
                                    pv[:, :FPC],
                                    xT[:, kc, st * P : (st + 1) * P],
                                    w_r[:, kc, :],
                                    start=(kc == 0),
                                    stop=(kc == KC - 1),
                                )
                            nc.vector.tensor_copy(
                                v_sb[:, blk * (SBLK // P) + st, :], pv[:, :FPC]
                            )

            # ---- phase 2: attention per q-block and head ----
            for qb in range(n_qb):
                n_qt = 4  # q-tiles per block
                aoT = aop.tile([64, HPC, QB], f32r, tag="aoT")
                for h in range(HPC):
                    hp = 64 * (h % 2)
                    hc = h // 2
                    attnT = attp.tile([P, NST, QB], f32r, tag="attnT")
                    if causal:
                        for jl in range(1, n_qt):
                            nc.vector.tensor_copy(
                                attnT[:, qb * n_qt + jl, 0 : jl * P],
                                zeros[:, 0 : jl * P],
                            )
                    for i in range(n_qt):
                        qi = qb * n_qt + i
                        prefix = (qi + 1) * P if causal else S
                        # ks chunks: full 512s then remainder
                        chunks = []
                        st = 0
                        while st < prefix:
                            w = min(512, prefix - st)
                            chunks.append((st, w))
                            st += w
                        sums = small.tile([P, 8], f32, tag="sums")
                        att = attrow.tile([P, S], f32, tag="att")
                        if not causal:
                            am = attrow.tile([P, S], f32, tag="am")
                            nc.sync.dma_start(
                                am[:], amask[qi * P : (qi + 1) * P, :]
                            )
                        for ci, (st, w) in enumerate(chunks):
                            ps = psA.tile([P, QB], f32, tag="A")
                            nc.tensor.matmul(
                                ps[:, :w],
                                qT[hp : hp + HD, hc, qi * P : (qi + 1) * P],
                                kT[hp : hp + HD, hc, st : st + w],
                                start=True,
                                stop=True,
                            )
                            if causal:
                                if st + w == prefix:  # diag block is chunk tail
                                    d0 = prefix - P - st
                                    nc.vector.tensor_tensor(
                                        ps[:, d0 : d0 + P],
                                        ps[:, d0 : d0 + P],
                                        cmask[:],
                                        mybir.AluOpType.add,
                                    )
                            else:
                                nc.vector.tensor_tensor(
                                    ps[:, :w],
                                    ps[:, :w],
                                    am[:, st : st + w],
                                    mybir.AluOpType.add,
                                )
                            nc.scalar.activation(
                                att[:, st : st + w],
                                ps[:, :w],
                                AF.Exp,
                                scale=1.0 / np.sqrt(HD),
                                accum_out=sums[:, ci : ci + 1],
                            )
                        rc = small.tile([P, 2], f32, tag="rc")
                        if len(chunks) > 1:
                            nc.vector.tensor_reduce(
                                rc[:, 0:1],
                                sums[:, : len(chunks)],
                                mybir.AxisListType.X,
                                mybir.AluOpType.add,
                            )
                        else:
                            nc.vector.tensor_copy(rc[:, 0:1], sums[:, 0:1])
                        nc.vector.reciprocal(rc[:, 1:2], rc[:, 0:1])
                        nc.vector.tensor_scalar_mul(
                            att[:, :prefix], att[:, :prefix], rc[:, 1:2]
                        )
                        nc.sync.dma_start(
                            attn_d[h, qi * P : (qi + 1) * P, 0:prefix],
                            att[:, :prefix],
                        )
                        z0 = prefix
                        while z0 < S:
                            zw = min(S // 2, S - z0)
                            nc.sync.dma_start(
                                attn_d[h, qi * P : (qi + 1) * P, z0 : z0 + zw],
                                zeros[:, :zw],
                            )
                            z0 += zw
                        for j in range(prefix // P):
                            pt = psB.tile([P, P], f32, tag="pt")
                            nc.tensor.transpose(
                                pt[:], att[:, j * P : (j + 1) * P], ident[:]
                            )
                            nc.vector.tensor_copy(
                                attnT[:, j, i * P : (i + 1) * P], pt[:]
                            )
                    # AV for this head over the whole q-block
                    njs = (qb + 1) * n_qt if causal else NST
                    pav = psC.tile([64, QB], f32, tag="av")
                    for j in range(njs):
                        nc.tensor.matmul(
                            pav[:],
                            v_sb[:, j, h * HD : (h + 1) * HD],
                            attnT[:, j, :],
                            start=(j == 0),
                            stop=(j == njs - 1),
                        )
                    nc.vector.tensor_copy(aoT[:, h, :], pav[:])
                # output projection for this q-block
                for i in range(n_qt):
                    qi = qb * n_qt + i
                    osb = osbp.tile([P, DIM], f32, tag="osb")
                    for half in range(2):
                        po = psA.tile([P, QB], f32, tag="A")
                        for c in range(HPC):
                            nc.tensor.matmul(
                                po[:],
                                aoT[:, c, i * P : (i + 1) * P],
                                wo_r[:, c, half * QB : (half + 1) * QB],
                                start=(c == 0),
                                stop=(c == HPC - 1),
                            )
                        nc.scalar.activation(
                            osb[:, half * QB : (half + 1) * QB], po[:], AF.Copy
                        )
                    nc.sync.dma_start(out_d[qi * P : (qi + 1) * P, :], osb[:])

    nc.compile()
    return nc


# test/profiling hooks (harmless defaults for grading)
TRACE = False
LAST_EXEC_NS = None


def kernel(query, key, value, mask, Wq, bq, Wk, bk, Wv, bv, Wo, bo):
    from concourse.bass_utils import run_bass_kernel_spmd

    query = np.asarray(query, np.float32)
    key = np.asarray(key, np.float32)
    value = np.asarray(value, np.float32)
    mask = np.asarray(mask)
    Wq = np.asarray(Wq, np.float32)
    bq = np.asarray(bq, np.float32)
    Wk = np.asarray(Wk, np.float32)
    bk = np.asarray(bk, np.float32)
    Wv = np.asarray(Wv, np.float32)
    bv = np.asarray(bv, np.float32)
    Wo = np.asarray(Wo, np.float32)
    bo = np.asarray(bo, np.float32)

    m2 = mask.reshape(S, S)
    causal = bool(np.array_equal(m2 != 0, np.tril(np.ones((S, S), bool))))
    nc = _build(causal)

    in_maps = []
    for c in range(NCORES):
        b = c // GROUPS
        hs = (c % GROUPS) * FPC
        he = hs + FPC
        im = {
            "xq": query[b],
            "xk": key[b],
            "xv": value[b],
            "wq": np.ascontiguousarray(Wq[hs:he, :].T),
            "wk": np.ascontiguousarray(Wk[hs:he, :].T),
            "wv": np.ascontiguousarray(Wv[hs:he, :].T),
            "wo": np.ascontiguousarray(Wo[:, hs:he].T),
            "bqk": np.stack(
                [
                    bq[hs : hs + P],
                    bq[hs + P : he],
                    bk[hs : hs + P],
                    bk[hs + P : he],
                ],
                axis=1,
            ).astype(np.float32),
        }
        if not causal:
            im["amask"] = np.where(m2 == 0, np.float32(-1e9), np.float32(0.0))
        in_maps.append(im)

    res = run_bass_kernel_spmd(nc, in_maps, list(range(NCORES)), trace=TRACE)
    global LAST_EXEC_NS
    LAST_EXEC_NS = res.exec_time_ns

    attn = np.empty((B, H, S, S), np.float32)
    out = np.zeros((B, S, DIM), np.float32)
    for c in range(NCORES):
        b = c // GROUPS
        hg = c % GROUPS
        r = res.results[c]
        attn[b, hg * HPC : (hg + 1) * HPC] = r["attn"]
        out[b] += r["out_part"]
    out += bv @ Wo.T + bo
    return out, attn
